# revision 19
# baseline (speedup 1.0000x reference)
"""Trainium2 Bass kernel for nn_CrossAttention_47004122087816.

Math (faithful to the reference's "buggy einsum"):
    xn   = LayerNorm(x); xnb = xn * ln_w + ln_b
    q    = (xnb @ Wq) * SCALE            [n, E]
    k, v = split(media @ Wkv)            [m, E] each
    sim  = q @ k^T                       [n, m]
    colsum[j] = sum_i softmax(sim, -1)[i, j]
    out[j, :] = colsum[j] * (v @ Wout)[j, :]

Sharding: pure data-parallel - batch b=8 over 8 NeuronCores.

Key optimizations over the original baseline:
 - Host casts x/media/weights to bf16 (halves HBM traffic) and pre-permutes
   weight rows so every HBM load has 8-16KB contiguous per-partition
   descriptors.  x/media carry rows 16p+u on partition p (coalesced loads);
   softmax rows are independent and colsum sums over all rows, so the row
   permutation needs no undo - the output store uses the same pattern.
   Output is stored bf16 and upcast on host.
 - All bulk HBM traffic runs on the gpsimd SWDGE queue (a single SWDGE
   dma_start spreads across all 16 DMA engines; HWDGE queues run DMAs with
   poor concurrency), ordered by when the pipeline needs each tensor:
   media c0 -> Wkv -> media/x alternating.  Small weights on scalar HWDGE.
 - PE emission order kv0,kv1,q0,kv2,q1,kv3,q2,q3 hides the longer q-path
   latency (load -> LN -> transpose) behind kv matmuls.
 - ln_w*SCALE folded into Wq on host; ln_b folded into a host-computed q0
   row added during the q PSUM evacuation (ScalarE, free).
 - exp runs on ScalarE with accum_out giving the softmax row-sum z for
   free; Y-matmul and colsum groups are interleaved between sim groups so
   semaphore latency never idles the PE.
 - colsum accumulates into four [1,512] PSUM rows packed at partition
   offsets 0/32/64/96 of a single PSUM bank (tile_position), leaving 7
   banks for matmul double-buffering.
 - final out[j,:] = c_j * Y[j,:] with Y = v @ Wout computed during the sim
   phase (c-independent); the tail is scatter + out-of-place scale (DVE /
   ScalarE; in-place DVE ops are pathologically slow) + SWDGE stores.
"""

import sys

for _p in ("/opt/trn_rl_repo",):
    if _p not in sys.path:
        sys.path.insert(0, _p)

import numpy as np
import ml_dtypes

import concourse.bass as bass  # noqa: F401
import concourse.tile as tile
from concourse import bacc, mybir
from concourse.bass_utils import run_bass_kernel_spmd

B = 8
N = 2048          # x rows per batch element
M = 2048          # media rows per batch element
D = 1024          # model dim
E = 512           # inner dim
P = 128           # partitions
F = 512           # one PSUM bank of fp32
CT = D // P       # 8  c-tiles (contraction over model dim)
ET = E // P       # 4  e-tiles (contraction over inner dim)
NT = N // P       # 16 row tiles
JC = M // F       # 4  column chunks of 512
RPP = N // P      # 16 rows per partition (coalesced DRAM layout)
SCALE = 64 ** -0.5
EPS = 1e-5

FP = mybir.dt.float32
BF = mybir.dt.bfloat16

AF = mybir.ActivationFunctionType
ALU = mybir.AluOpType
AX = mybir.AxisListType


def _build():
    nc = bacc.Bacc("TRN2", target_bir_lowering=False, debug=False, num_devices=B)

    x = nc.dram_tensor("x", [N, D], BF, kind="ExternalInput").ap()
    media = nc.dram_tensor("media", [M, D], BF, kind="ExternalInput").ap()
    # weights pre-permuted on host: row (p*CT + kt) holds original row (kt*P + p)
    wq = nc.dram_tensor("wq", [D, E], BF, kind="ExternalInput").ap()
    wkv = nc.dram_tensor("wkv", [D, 2 * E], BF, kind="ExternalInput").ap()
    wout = nc.dram_tensor("wout", [E, D], BF, kind="ExternalInput").ap()
    q0 = nc.dram_tensor("q0", [P, ET], FP, kind="ExternalInput").ap()
    out = nc.dram_tensor("out", [M, D], BF, kind="ExternalOutput").ap()

    with tile.TileContext(nc) as tc:
        from contextlib import ExitStack

        with ExitStack() as ctx:
            consts = ctx.enter_context(tc.tile_pool(name="consts", bufs=1))
            acts = ctx.enter_context(tc.tile_pool(name="acts", bufs=1))
            xst = ctx.enter_context(tc.tile_pool(name="xst", bufs=2))
            mst = ctx.enter_context(tc.tile_pool(name="mst", bufs=2))
            obuf = ctx.enter_context(tc.tile_pool(name="obuf", bufs=2))
            xhp = ctx.enter_context(tc.tile_pool(name="xhp", bufs=2))
            xw = ctx.enter_context(tc.tile_pool(name="xw", bufs=2))
            mtw = ctx.enter_context(tc.tile_pool(name="mtw", bufs=2))
            expp = ctx.enter_context(tc.tile_pool(name="expp", bufs=2))
            zp = ctx.enter_context(tc.tile_pool(name="zp", bufs=3))
            small = ctx.enter_context(tc.tile_pool(name="small", bufs=6))
            psim = ctx.enter_context(tc.tile_pool(name="psim", bufs=4, space="PSUM"))
            psy = ctx.enter_context(tc.tile_pool(name="psy", bufs=3, space="PSUM"))
            pscs = ctx.enter_context(tc.tile_pool(name="pscs", bufs=1, space="PSUM"))

            wkv_t = consts.tile([P, CT, 2 * E], BF)
            wq_t = consts.tile([P, CT, E], BF)
            wout_t = consts.tile([P, ET, D], BF)
            q0t = consts.tile([P, ET], FP)
            eps_t = consts.tile([P, 1], FP)
            nc.vector.memset(eps_t[:], EPS)

            kT = acts.tile([P, ET, M], BF)
            vT = acts.tile([P, ET, M], BF)
            qT = acts.tile([P, ET, N], BF)
            Y = acts.tile([P, RPP * D], BF)
            scol = consts.tile([P, NT], FP)

            xv = x.rearrange("(p t) d -> p t d", t=RPP)
            mv = media.rearrange("(p t) d -> p t d", t=RPP)
            ov = out.rearrange("(p t) d -> p t d", t=RPP)

            # ------------- bulk loads: gpsimd SWDGE in need-order ------------
            mstc: list = [None] * JC
            xstc: list = [None] * JC

            def load4(kind, c):
                if kind == "m":
                    t = mst.tile([P, 4 * D], BF, tag="mst", name=f"mst{c}")
                    nc.gpsimd.dma_start(t[:], mv[:, 4 * c : 4 * c + 4, :])
                    mstc[c] = t
                else:
                    t = xst.tile([P, 4 * D], BF, tag="xst", name=f"xst{c}")
                    nc.gpsimd.dma_start(t[:], xv[:, 4 * c : 4 * c + 4, :])
                    xstc[c] = t

            load4("m", 0)
            nc.gpsimd.dma_start(
                wkv_t[:], wkv.rearrange("(p kt) e -> p kt e", kt=CT)
            )
            load4("m", 1)
            load4("x", 0)
            load4("m", 2)
            load4("x", 1)
            load4("m", 3)
            load4("x", 2)
            load4("x", 3)
            # small weights on the otherwise-idle scalar HWDGE queue
            nc.scalar.dma_start(wq_t[:], wq.rearrange("(p kt) e -> p kt e", kt=CT))
            nc.scalar.dma_start(q0t[:], q0)
            nc.scalar.dma_start(
                wout_t[:], wout.rearrange("(p et) d -> p et d", et=ET)
            )

            # ---------------- feed helpers -----------------------------------
            def ln_block(xin, name):
                st = small.tile([P, 2, 6], FP, tag="st", name=f"st{name}")
                for sg in range(2):
                    nc.vector.bn_stats(st[:, sg, :], xin[:, sg * 512 : (sg + 1) * 512])
                mvt = small.tile([P, 2], FP, tag="mv", name=f"mv{name}")
                nc.vector.bn_aggr(mvt[:], st[:])
                sd = small.tile([P, 1], FP, tag="sd", name=f"sd{name}")
                nc.scalar.activation(
                    sd[:], mvt[:, 1:2], func=AF.Sqrt, bias=eps_t[:], scale=1.0
                )
                rsig = small.tile([P, 1], FP, tag="rsig", name=f"rsig{name}")
                nc.vector.reciprocal(rsig[:], sd[:])
                nmr = small.tile([P, 1], FP, tag="nmr", name=f"nmr{name}")
                nc.vector.tensor_scalar(
                    nmr[:], mvt[:, 0:1], rsig[:], -1.0, ALU.mult, ALU.mult
                )
                xh = xhp.tile([P, D], BF, tag="xh", name=f"xh{name}")
                nc.scalar.activation(
                    xh[:], xin[:], func=AF.Identity, bias=nmr[:], scale=rsig[:]
                )
                return xh

            def m_transpose(c):
                mtw_c = mtw.tile([P, CT, F], BF, tag="mtw", name=f"mtw{c}")
                for u in range(4):
                    nc.sync.dma_start_transpose(
                        mtw_c[:, :, u * P : (u + 1) * P],
                        mstc[c][:, u * D : (u + 1) * D],
                    )
                return mtw_c

            def x_transpose(c):
                xw_c = xw.tile([P, CT, F], BF, tag="xw", name=f"xw{c}")
                for u in range(4):
                    xh = ln_block(xstc[c][:, u * D : (u + 1) * D], f"{c}_{u}")
                    nc.sync.dma_start_transpose(
                        xw_c[:, :, u * P : (u + 1) * P], xh[:]
                    )
                return xw_c

            def kv_chunk(c, mtw_c):
                for ph in range(4):  # (k e01) (k e23) (v e01) (v e23)
                    for half in range(2):
                        e = ph * 2 + half
                        ps = psim.tile([P, F], FP, tag="ps", name=f"kv{c}_{e}")
                        for kt in range(CT):
                            nc.tensor.matmul(
                                ps[:],
                                lhsT=wkv_t[:, kt, e * P : (e + 1) * P],
                                rhs=mtw_c[:, kt, :],
                                start=(kt == 0),
                                stop=(kt == CT - 1),
                            )
                        if ph < 2:  # k
                            nc.scalar.copy(kT[:, e, c * F : (c + 1) * F], ps[:])
                        else:  # v
                            nc.vector.tensor_copy(
                                vT[:, e - 4, c * F : (c + 1) * F], ps[:]
                            )

            def q_chunk(c, xw_c):
                for dt in range(ET):
                    ps = psim.tile([P, F], FP, tag="ps", name=f"q{c}_{dt}")
                    for kt in range(CT):
                        nc.tensor.matmul(
                            ps[:],
                            lhsT=wq_t[:, kt, dt * P : (dt + 1) * P],
                            rhs=xw_c[:, kt, :],
                            start=(kt == 0),
                            stop=(kt == CT - 1),
                        )
                    nc.scalar.activation(
                        qT[:, dt, c * F : (c + 1) * F],
                        ps[:],
                        func=AF.Identity,
                        bias=q0t[:, dt : dt + 1],
                        scale=1.0,
                    )

            # ------- feed: PE order kv0,kv1,q0,kv2,q1,kv3,q2,q3 --------------
            mtw0 = m_transpose(0)
            mtw1 = m_transpose(1)
            xw0 = x_transpose(0)
            kv_chunk(0, mtw0)
            kv_chunk(1, mtw1)
            xw1 = x_transpose(1)
            q_chunk(0, xw0)
            mtw2 = m_transpose(2)
            kv_chunk(2, mtw2)
            xw2 = x_transpose(2)
            q_chunk(1, xw1)
            mtw3 = m_transpose(3)
            kv_chunk(3, mtw3)
            xw3 = x_transpose(3)
            q_chunk(2, xw2)
            q_chunk(3, xw3)

            # ---------------- sim, exp (+z via accum), colsum, Y -------------
            # colsum rows live at partition offsets 0/32/64/96 of ONE bank
            cs_all = pscs.tile([P, F], FP)
            ex_hist: list = [None, None]
            zrb_hist: list = [None, None]

            def colsum_mms(it):
                ex_t = ex_hist[it % 2]
                zrb_t = zrb_hist[it % 2]
                for jc in range(JC):
                    nc.tensor.matmul(
                        cs_all[32 * jc : 32 * jc + 1, :],
                        lhsT=zrb_t[:],
                        rhs=ex_t[:, jc * F : (jc + 1) * F],
                        start=(it == 0),
                        stop=(it == NT - 1),
                        skip_group_check=True,
                        tile_position=(0, 32 * jc),
                    )

            def sim_group(it, jc, ex, zpart):
                ps = psim.tile([P, F], FP, tag="ps", name=f"sim{it}_{jc}")
                for et in range(ET):
                    nc.tensor.matmul(
                        ps[:],
                        lhsT=qT[:, et, it * P : (it + 1) * P],
                        rhs=kT[:, et, jc * F : (jc + 1) * F],
                        start=(et == 0),
                        stop=(et == ET - 1),
                    )
                nc.scalar.activation(
                    ex[:, jc * F : (jc + 1) * F],
                    ps[:],
                    func=AF.Exp,
                    bias=0.0,
                    scale=1.0,
                    accum_out=zpart[:, jc : jc + 1],
                )

            def y_group(it, n2):
                psn = psy.tile([P, F], FP, tag="py", name=f"y{it}_{n2}")
                for et in range(ET):
                    nc.tensor.matmul(
                        psn[:],
                        lhsT=vT[:, et, it * P : (it + 1) * P],
                        rhs=wout_t[:, et, n2 * F : (n2 + 1) * F],
                        start=(et == 0),
                        stop=(et == ET - 1),
                    )
                nc.vector.tensor_copy(
                    Y[:, it * D + n2 * F : it * D + (n2 + 1) * F], psn[:]
                )

            for it in range(NT):
                ex = expp.tile([P, M], BF, tag="ex", name=f"ex{it}")
                zpart = small.tile([P, JC], FP, tag="zpt", name=f"zpt{it}")
                sim_group(it, 0, ex, zpart)
                sim_group(it, 1, ex, zpart)
                y_group(it, 0)
                sim_group(it, 2, ex, zpart)
                if it > 0:
                    colsum_mms(it - 1)
                sim_group(it, 3, ex, zpart)
                y_group(it, 1)
                z = small.tile([P, 1], FP, tag="z", name=f"z{it}")
                nc.vector.tensor_reduce(z[:], zpart[:], axis=AX.X, op=ALU.add)
                zr = small.tile([P, 1], FP, tag="zr", name=f"zr{it}")
                nc.vector.reciprocal(zr[:], z[:])
                zrb = zp.tile([P, 1], BF, tag="zrb", name=f"zrb{it}")
                nc.vector.tensor_copy(zrb[:], zr[:])
                ex_hist[it % 2] = ex
                zrb_hist[it % 2] = zrb
            colsum_mms(NT - 1)

            # ---------------- tail: scatter colsum, scale Y, store -----------
            # PSUM is not DMA-readable: one whole-bank copy to SBUF first
            # (only partitions 0/32/64/96 are meaningful), then single-column
            # scatters scol[p, jc*4+b] = cs[32*jc, b*128+p].
            csum_sb = consts.tile([P, F], FP)
            nc.scalar.copy(csum_sb[:], cs_all[:])
            for jt in range(NT):
                jc, b = jt // 4, jt % 4
                q = nc.sync if jt % 2 == 0 else nc.scalar
                q.dma_start(
                    scol[:, jt : jt + 1],
                    csum_sb[32 * jc : 32 * jc + 1, b * P : (b + 1) * P],
                )
            # out-of-place scales (in-place DVE tensor ops are ~20x slower on
            # HW) alternating DVE / ScalarE, store per tile-pair over SWDGE
            for s in range(NT // 2):
                ob = obuf.tile([P, 2 * D], BF, tag="ob", name=f"ob{s}")
                for h in range(2):
                    jt = 2 * s + h
                    ysl = Y[:, jt * D : (jt + 1) * D]
                    osl = ob[:, h * D : (h + 1) * D]
                    csl = scol[:, jt : jt + 1]
                    if jt % 2 == 0:
                        nc.vector.tensor_scalar_mul(osl, ysl, csl)
                    else:
                        nc.scalar.mul(osl, ysl, csl)
                nc.gpsimd.dma_start(ov[:, 2 * s : 2 * s + 2, :], ob[:])

    nc.compile()
    return nc


_NC_CACHE = None


def _get_nc():
    global _NC_CACHE
    if _NC_CACHE is None:
        _NC_CACHE = _build()
    return _NC_CACHE


BF_NP = ml_dtypes.bfloat16


def _run(inputs, trace=False, **kw):
    nc = _get_nc()
    ln_w = np.asarray(inputs["ln_w"], dtype=np.float32)
    ln_b = np.asarray(inputs["ln_b"], dtype=np.float32)
    Wq = np.asarray(inputs["Wq"], dtype=np.float32)
    Wkv = np.asarray(inputs["Wkv"], dtype=np.float32)
    Wout = np.asarray(inputs["Wout"], dtype=np.float32)

    def permute_rows(w):  # row (kt*P + p) -> row (p*ct + kt) for big packets
        ct = w.shape[0] // P
        return np.ascontiguousarray(
            w.reshape(ct, P, w.shape[1]).transpose(1, 0, 2).reshape(w.shape)
        )

    wq_h = permute_rows((Wq * (SCALE * ln_w)[:, None]).astype(BF_NP))
    wkv_h = permute_rows(Wkv.astype(BF_NP))
    wout_h = permute_rows(Wout.astype(BF_NP))
    q0_h = np.ascontiguousarray(
        (SCALE * (ln_b @ Wq)).astype(np.float32).reshape(ET, P).T
    )

    xs = np.asarray(inputs["x"], dtype=np.float32).astype(BF_NP)
    ms = np.asarray(inputs["media"], dtype=np.float32).astype(BF_NP)
    shared = {"wq": wq_h, "wkv": wkv_h, "wout": wout_h, "q0": q0_h}
    in_maps = [
        dict(shared, x=np.ascontiguousarray(xs[b]), media=np.ascontiguousarray(ms[b]))
        for b in range(B)
    ]
    res = run_bass_kernel_spmd(nc, in_maps, core_ids=list(range(B)), trace=trace, **kw)
    out = np.stack(
        [res.results[b]["out"].astype(np.float32) for b in range(B)], axis=0
    )
    return out, res


def kernel(**inputs) -> np.ndarray:
    out, _ = _run(inputs, trace=False)
    return out
